# revision 5
# baseline (speedup 1.0000x reference)
"""Trainium2 Bass kernel for nn_AttentionMemory (sparse_attention).

reference:
    mkf = mk.reshape(B, CK, HW); qkf = qk.reshape(B, CK, HW)
    affinity[b, m, q] = (-|mk_m|^2 + 2 mk_m.qk_q - |qk_q|^2) / sqrt(CK)
    out = softmax(affinity, axis=m)

Transposed compute (q on partitions, m on the free dim), so the softmax
reduction runs along the free dim and the PE never does sum-matmuls:

    z[q, m] = qk_q.mk_m - |mk_m|^2/2         (augmented K=65 matmul, f32r)
    out_t[q, m] = exp(z/4) / sum_m exp(z/4)  (= out[m, q]; host transposes)

Per (batch, q-tile of 128) the m axis is split 2048 + 256:
  - main sweep: 4 matmuls N=512 into a 4-bank PSUM tile; ONE Exp activation
    over [128, 2048] with accum_out giving the row-sum for free.
  - remainder sweep: eight tiles' [128, 256] remainders are batched into one
    4-bank PSUM tile and ONE [128, 2048] Exp; a small per-tile DVE reduce
    supplies the missing 256-column partial sum.
The Activation engine is the bottleneck (~90 us/core of the ~100 us total:
83k exp elements/lane at 1.2 GHz is a 69 us floor, plus ~260 ns/instruction
overhead and ~220 ns per accumulator read, both HW-measured).  The kernel
keeps ACT saturated: the three rem groups run at each batch head so their
full-width exps cover the main-sweep prep latency; every other engine (PE
~40 us, DVE ~60 us, DMA ~78 us incl. 21 MB of f16 stores, Pool ~20 us) is
scheduled under the ACT stream.  The bias row -|mk_m|^2/2 is built by a
selector matmul (ones in column CK -> PSUM partition 64) + one-partition
DVE copies; the gpsimd partition_all_reduce path measured ~5x the model
cost on HW and starved ACT at batch heads.  The normalization multiply
runs on DVE in 4x mode (bf16 in / f16 out, per-partition scalar 1/S);
stores go out as [128, 2304] f16 tiles alternating sync/gpsimd queues.

Distribution: data parallel over B=16 -> 2 batches/core on 8 NeuronCores.
kernel() transposes (q, m) -> (m, q) on the host and upcasts f16 -> f32.
"""

import numpy as np
from contextlib import ExitStack

import concourse.bass as bass
import concourse.tile as tile
from concourse import bacc, mybir

B, CK, H, W = 16, 64, 36, 64
HW = H * W                 # 2304
NCORES = 8
BL = B // NCORES           # 2 batches per core
QT = HW // 128             # 18 q-tiles
KA = CK + 1                # augmented K (ones row in qk, bias row in mk)
MM = 2048                  # main m-span (4 PSUM banks)
MR = HW - MM               # 256 remainder columns
F32 = mybir.dt.float32
F32R = mybir.dt.float32r
BF16 = mybir.dt.bfloat16
F16 = mybir.dt.float16
AF = mybir.ActivationFunctionType
ALU = mybir.AluOpType

# tiles whose row-sum (main part) comes from a DVE reduce instead of the
# activation's accum_out: lets us trade ACT accumulator-read time for DVE
# time when balancing the two engines on hardware.
DVE_SUM_TILES = ()


def _make_pools(tc, ctx):
    return dict(
        prep=ctx.enter_context(tc.tile_pool(name="prep", bufs=1)),
        e_pool=ctx.enter_context(tc.tile_pool(name="e_sb", bufs=1)),
        o_pool=ctx.enter_context(tc.tile_pool(name="o_sb", bufs=4)),
        s_pool=ctx.enter_context(tc.tile_pool(name="s", bufs=2)),
        z_pool=ctx.enter_context(tc.tile_pool(name="z", bufs=2, space="PSUM")),
    )


def _alloc_aug(pool, b, slot=None):
    # batch-0 tiles are carried across loop bodies (prefetch); they use two
    # explicitly alternating single-buffer tags so each body's buffer
    # assignment is static and back-edge consistent (period 2 divides the
    # 8-body unroll).  batch-1 tiles live within one body; a plain 2-deep
    # ring tag suffices.
    tag = f"{b}" if slot is None else f"{b}s{slot}"
    return (
        pool.tile([KA, HW], F32R, name=f"mk_aug{b}", tag=f"mk{tag}"),
        pool.tile([KA, HW], F32R, name=f"qk_aug{b}", tag=f"qk{tag}"),
    )


def _emit_prep(tc, nc, pools, mk_ext, qk_ext, ones_dram, sel, b, tiles,
               qk_split):
    """Load + prepare one batch's augmented inputs.

    sq = -mk^2/2 (DVE); the bias row sum_c sq lands on PSUM partition 64 via
    a selector matmul (ones in column CK only) and a cheap one-partition DVE
    copy moves it onto the augmented row 64 that the matmuls consume.  The
    gpsimd path (partition_all_reduce + Q7 copy) measured ~5x the model cost
    on hardware and starved the ACT stream at every batch head.  Remainder
    columns go first: they feed the rem-group matmuls that give ACT its
    first full-size spans.
    """
    mk_flat = mk_ext.rearrange("b c n -> (b c) n")
    qk_flat = qk_ext.rearrange("b c n -> (b c) n")
    mk_t, qk_t = tiles
    prep = pools["prep"]
    z_pool = pools["z_pool"]
    sq = prep.tile([CK, HW], F32R, name=f"sq_{b}", tag="sq")
    mkf = mk_flat[b * CK : (b + 1) * CK, :]
    qkf = qk_flat[b * CK : (b + 1) * CK, :]

    def chunk(a_ps, o0, c0, cw):
        with tc.high_priority(offset=900):
            nc.gpsimd.dma_start(mk_t[0:CK, c0 : c0 + cw], mkf[:, c0 : c0 + cw])
        nc.vector.scalar_tensor_tensor(
            out=sq[:, c0 : c0 + cw],
            in0=mk_t[0:CK, c0 : c0 + cw].bitcast(F32),
            scalar=-0.5,
            in1=mk_t[0:CK, c0 : c0 + cw].bitcast(F32),
            op0=ALU.mult,
            op1=ALU.mult,
        )
        nc.tensor.matmul(
            a_ps[:, o0 : o0 + cw],
            lhsT=sel[:],
            rhs=sq[:, c0 : c0 + cw],
            start=True,
            stop=True,
        )
        nc.vector.tensor_copy(
            mk_t[CK : CK + 1, c0 : c0 + cw], a_ps[CK : CK + 1, o0 : o0 + cw]
        )

    with tc.high_priority(offset=900):
        nc.gpsimd.dma_start(qk_t[CK : CK + 1, :], ones_dram[:])
    a_psB = z_pool.tile([128, MM], F32, name=f"apB_{b}", tag="z")
    chunk(a_psB, 0, MM, MR)
    if qk_split:
        with tc.high_priority(offset=900):
            nc.gpsimd.dma_start(qk_t[0:CK, 0:1024], qkf[:, 0:1024])
            nc.gpsimd.dma_start(qk_t[0:CK, 1024:], qkf[:, 1024:])
    else:
        with tc.high_priority(offset=900):
            nc.gpsimd.dma_start(qk_t[0:CK, :], qkf[:])
    a_psA = z_pool.tile([128, MM], F32, name=f"apA_{b}", tag="z")
    for c in range(4):
        chunk(a_psA, c * 512, c * 512, 512)


def _build_kernel(tc: tile.TileContext, out_ext, mk_ext, qk_ext, aug_pool,
                  aug0_pool, ones_dram, sel, pools, e_sb):
    nc = tc.nc
    if True:
        o_pool = pools["o_pool"]
        s_pool = pools["s_pool"]
        z_pool = pools["z_pool"]

        s1 = [s_pool.tile([128, QT], F32, name=f"s1_{b}", tag="s1") for b in range(BL)]
        s2 = [s_pool.tile([128, QT], F32, name=f"s2_{b}", tag="s2") for b in range(BL)]
        st = [s_pool.tile([128, QT], F32, name=f"st_{b}", tag="st") for b in range(BL)]
        rr = [s_pool.tile([128, QT], F32, name=f"rr_{b}", tag="rr") for b in range(BL)]
        _dma_rr = [0]

        def emit_rem_group(b, tiles, g):
            mk_t, qk_t = tiles
            t0, t1 = 8 * g, min(8 * g + 8, QT)
            n = t1 - t0
            zr = z_pool.tile([128, MM], F32, name=f"zr_{b}_{g}", tag="z")
            for i, t in enumerate(range(t0, t1)):
                nc.tensor.matmul(
                    zr[:, i * MR : (i + 1) * MR],
                    lhsT=qk_t[:, t * 128 : (t + 1) * 128],
                    rhs=mk_t[:, MM:HW],
                    start=True,
                    stop=True,
                )
            nc.scalar.activation(
                e_sb[:, t0:t1, MM:HW],
                zr[:, 0 : n * MR].rearrange("p (u q) -> p u q", u=n),
                AF.Exp,
                scale=0.25,
            )

        def emit_main(b, tiles, t):
            mk_t, qk_t = tiles
            z = z_pool.tile([128, MM], F32, name=f"z_{b}_{t}", tag="z")
            for c in range(4):
                nc.tensor.matmul(
                    z[:, c * 512 : (c + 1) * 512],
                    lhsT=qk_t[:, t * 128 : (t + 1) * 128],
                    rhs=mk_t[:, c * 512 : (c + 1) * 512],
                    start=True,
                    stop=True,
                )
            if t in DVE_SUM_TILES:
                nc.scalar.activation(e_sb[:, t, 0:MM], z[:], AF.Exp, scale=0.25)
                nc.vector.tensor_reduce(
                    s1[b][:, t : t + 1], e_sb[:, t, 0:MM], mybir.AxisListType.X,
                    ALU.add,
                )
            else:
                nc.scalar.activation(
                    e_sb[:, t, 0:MM], z[:], AF.Exp, scale=0.25,
                    accum_out=s1[b][:, t : t + 1],
                )

        def emit_fin(b, t, split_store=False):
            nc.vector.tensor_reduce(
                s2[b][:, t : t + 1], e_sb[:, t, MM:HW], mybir.AxisListType.X,
                ALU.add,
            )
            nc.vector.tensor_tensor(
                out=st[b][:, t : t + 1],
                in0=s1[b][:, t : t + 1],
                in1=s2[b][:, t : t + 1],
                op=ALU.add,
            )
            nc.vector.reciprocal_approx_fast(
                rr[b][:, t : t + 1], st[b][:, t : t + 1]
            )
            o_sb = o_pool.tile([128, HW], F16, name="o_sb", tag="o")
            nc.vector.tensor_scalar_mul(
                o_sb[:], e_sb[:, t, :], rr[b][:, t : t + 1]
            )
            dest = out_ext[b, t * 128 : (t + 1) * 128, :]
            if split_store:
                # final tile: halve the store across both queues so the
                # end-of-kernel drain is short
                nc.sync.dma_start(dest[:, 0:1152], o_sb[:, 0:1152])
                nc.gpsimd.dma_start(dest[:, 1152:], o_sb[:, 1152:])
            else:
                eng = (nc.sync, nc.gpsimd)[_dma_rr[0] % 2]
                _dma_rr[0] += 1
                eng.dma_start(dest, o_sb[:])

        b0_tiles = _alloc_aug(aug0_pool, 0)
        _emit_prep(tc, nc, pools, mk_ext, qk_ext, ones_dram, sel, 0,
                   b0_tiles, qk_split=True)
        b1_tiles = None
        for b in range(BL):
            tiles = b0_tiles if b == 0 else b1_tiles
            # all three rem groups run at the head of the batch: they only
            # need the remainder prep chunk, so their full-width exps keep
            # ACT busy while the main-sweep prep finishes; emitting them
            # before any main keeps them ahead of main0 in the ACT FIFO
            for g in range(3):
                emit_rem_group(b, tiles, g)
            for t in range(QT):
                emit_main(b, tiles, t)
                if t == 3 and b == 0:
                    b1_tiles = _alloc_aug(aug_pool, 1)
                    _emit_prep(tc, nc, pools, mk_ext, qk_ext, ones_dram,
                               sel, 1, b1_tiles, qk_split=False)
                if t >= 1:
                    emit_fin(b, t - 1)
            emit_fin(b, QT - 1, split_store=(b == BL - 1))


_CACHE = {}


def _get_compiled(niter: int = 1):
    """Build+compile the per-core graph. niter>1 wraps the body in a For_i
    hardware loop (identical I/O each iteration) for differential timing."""
    key = ("nc", niter)
    if key not in _CACHE:
        nc = bacc.Bacc("TRN2", target_bir_lowering=False, debug=False)
        mk_ext = nc.dram_tensor("mk", [BL, CK, HW], F32R, kind="ExternalInput").ap()
        qk_ext = nc.dram_tensor("qk", [BL, CK, HW], F32R, kind="ExternalInput").ap()
        out_ext = nc.dram_tensor("out", [BL, HW, HW], F16, kind="ExternalOutput").ap()
        ones_dram = nc.dram_tensor("ones_i", [1, HW], F32R, kind="Internal").ap()
        with tile.TileContext(nc) as tc:
            # stage a ones row in DRAM once; each loop body DMAs it onto the
            # augmented row 64 of qk (9 KB, negligible next to the inputs)
            with tc.tile_pool(name="init", bufs=1) as init_pool:
                # [128, 18] memset is a ~20-cycle DVE op; the DMA scatters it
                # into the flat [1, HW] DRAM row
                ones_t = init_pool.tile([128, QT], F32, name="ones_t")
                nc.vector.memset(ones_t[:], 1.0)
                nc.sync.dma_start(
                    ones_dram.rearrange("1 (p k) -> p k", p=128),
                    ones_t[:].bitcast(F32R),
                )
            with ExitStack() as pctx:
                aug_pool = pctx.enter_context(tc.tile_pool(name="aug", bufs=2))
                aug0_pool = pctx.enter_context(tc.tile_pool(name="aug0", bufs=3))
                singles = pctx.enter_context(tc.tile_pool(name="sing", bufs=1))
                pools = _make_pools(tc, pctx)
                e_sb = pools["e_pool"].tile([128, QT, HW], BF16, name="e_sb")
                # prologue: ACT table preload + cold prep of the first
                # iteration's batch 0 (each body then prefetches the next)
                dummy = singles.tile([1, 1], F32, name="dummy")
                nc.gpsimd.memset(dummy[:], 1.0)
                nc.scalar.activation(dummy[:], dummy[:], AF.Exp)
                # selector stationary: ones in column CK only, so the
                # bias-row matmul writes its sums onto PSUM partition 64
                sel = singles.tile([CK, 128], F32R, name="sel")
                sel_f32 = singles.tile([CK, 128], F32, name="sel_f32")
                nc.vector.memset(sel_f32[:], 0.0)
                nc.vector.memset(sel_f32[:, CK : CK + 1], 1.0)
                nc.vector.tensor_copy(sel[:], sel_f32[:])
                body = lambda _i=None: _build_kernel(
                    tc, out_ext, mk_ext, qk_ext, aug_pool, aug0_pool,
                    ones_dram, sel, pools, e_sb
                )
                if niter == 1:
                    body()
                elif niter < 0:
                    # plain unrolled repetition (no For_i): simulator-friendly
                    for _ in range(-niter):
                        body()
                else:
                    tc.For_i_unrolled(0, niter, 1, body, max_unroll=16)
        nc.compile()
        _CACHE[key] = nc
    return _CACHE[key]


class _CachedRunner:
    """Compile/upload the executable once; reuse the jitted callable for
    every subsequent call."""

    def __init__(self, nc, n_cores: int):
        import jax
        from jax.sharding import Mesh, PartitionSpec, NamedSharding
        from jax.experimental.shard_map import shard_map
        from concourse import bass2jax

        bass2jax.install_neuronx_cc_hook()
        self.n_cores = n_cores
        partition_name = (
            nc.partition_id_tensor.name if nc.partition_id_tensor else None
        )
        in_names, out_names, out_avals, zero_outs = [], [], [], []
        for alloc in nc.m.functions[0].allocations:
            if not isinstance(alloc, mybir.MemoryLocationSet):
                continue
            name = alloc.memorylocations[0].name
            if alloc.kind == "ExternalInput":
                if name != partition_name:
                    in_names.append(name)
            elif alloc.kind == "ExternalOutput":
                out_names.append(name)
                shape = tuple(alloc.tensor_shape)
                dtype = mybir.dt.np(alloc.dtype)
                out_avals.append(jax.core.ShapedArray(shape, dtype))
                zero_outs.append(np.zeros(shape, dtype))
        n_params = len(in_names)
        in_names = in_names + out_names
        if partition_name is not None:
            in_names.append(partition_name)
        self.in_names, self.out_names = in_names, out_names
        self.n_params, self.out_avals = n_params, out_avals

        def _body(*args):
            operands = list(args)
            if partition_name is not None:
                operands.append(bass2jax.partition_id_tensor())
            return tuple(
                bass2jax._bass_exec_p.bind(
                    *operands,
                    out_avals=tuple(out_avals),
                    in_names=tuple(in_names),
                    out_names=tuple(out_names),
                    lowering_input_output_aliases=(),
                    sim_require_finite=True,
                    sim_require_nnan=True,
                    nc=nc,
                )
            )

        P = PartitionSpec
        mesh = Mesh(np.asarray(jax.devices()[:n_cores]), ("core",))
        self.fn = jax.jit(
            shard_map(
                _body,
                mesh=mesh,
                in_specs=(P("core"),) * (n_params + len(out_names)),
                out_specs=(P("core"),) * len(out_names),
                check_rep=False,
            ),
            keep_unused=True,
        )
        sharding = NamedSharding(mesh, P("core"))
        self.zeros_dev = [
            jax.device_put(
                np.zeros((n_cores * z.shape[0], *z.shape[1:]), z.dtype), sharding
            )
            for z in zero_outs
        ]

    def __call__(self, in_maps):
        concat_in = [
            np.concatenate([np.asarray(m[name]) for m in in_maps], axis=0)
            for name in self.in_names[: self.n_params]
        ]
        out_arrs = self.fn(*concat_in, *self.zeros_dev)
        host = [
            np.asarray(a).reshape(self.n_cores, *self.out_avals[i].shape)
            for i, a in enumerate(out_arrs)
        ]
        return [
            {name: host[i][c] for i, name in enumerate(self.out_names)}
            for c in range(self.n_cores)
        ]


def _get_runner(niter: int = 1) -> "_CachedRunner":
    key = ("runner", niter)
    if key not in _CACHE:
        _CACHE[key] = _CachedRunner(_get_compiled(niter), NCORES)
    return _CACHE[key]


def run_spmd(mk: np.ndarray, qk: np.ndarray, niter: int = 1) -> np.ndarray:
    mk = np.ascontiguousarray(np.asarray(mk, dtype=np.float32).reshape(B, CK, HW))
    qk = np.ascontiguousarray(np.asarray(qk, dtype=np.float32).reshape(B, CK, HW))
    in_maps = [
        {"mk": mk[c * BL : (c + 1) * BL], "qk": qk[c * BL : (c + 1) * BL]}
        for c in range(NCORES)
    ]
    res = _get_runner(niter)(in_maps)
    out = np.concatenate([res[c]["out"] for c in range(NCORES)], axis=0)
    # device computed out_t[b, q, m]; reference wants out[b, m, q]
    return out.reshape(B, HW, HW).transpose(0, 2, 1)


def kernel(mk: np.ndarray, qk: np.ndarray) -> np.ndarray:
    return run_spmd(mk, qk, niter=1).astype(np.float32)
